# revision 23
# baseline (speedup 1.0000x reference)
"""Causal single-head attention on 8 TRN2 NeuronCores.

Problem: x[B=8,T=2048,E=1024] fp32, per-head Q/K/V projections (D=128) +
causal softmax attention. Sharding: data-parallel over batch B — one batch
element per core; Q/K/V weights replicated.

Per-core algorithm:

  x is fed pre-transposed as x8T [E, T] in fp8e4m3 so the contraction dim
  (E) lies on SBUF partitions and the input stream is 2MB instead of 4MB.
  1. qT/kT/vT [D, Tt] via fp8 DoubleRow matmuls: e-chunk PAIRS (K=256) per
     matmul at 0.5 cycles/row.  V adds a second accumulation chain against
     the fp8 weight residual rW8v so v == x8 @ Wv exactly up to fp8 input
     quantization of x.  Bias added during the PSUM->SBUF move (DVE
     tensor_scalar_add, fp16 out).
  2. v natural [s, D] chunks by PE-transposing vT 128x128 blocks, stored
     ones-augmented: v_aug [s, 129] with col 128 == 1.
  3. Scores TRANSPOSED in fp16: S^T tile [s 128, t 512] = kT_slice @ qT.
     exp via ACT (scale=1/sqrt(D)), fp16 out.  Causality: s-chunks above
     the diagonal skipped; diagonal chunks compute only the live column
     range; a 128x128 lower-triangle 0/1 multiply (gpsimd) masks the
     diagonal sub-block.
  4. PV natural per 128-row t-chunk: out_psum [t 128, 129] +=
     ex_slice-as-stationary @ v_aug_chunk; column 128 accumulates the
     softmax denominator via the ones column.
  5. No on-device normalize: numerator||denominator copied fp16 and DMA'd
     out [T, 129]; the host divides (and casts fp32).

The host also recomputes rows t < 64 exactly in fp32: for small softmax
support the fp8 x-quantization error is not averaged away, so those rows
are replaced with an exact host computation (cheap: 64 rows).

Edge scheduling: input DMAs ride one queue in exact consumption order
(W8q, x8-tile0 in halves, W8k, W8v+rW8v, x8-tile1..3) so each completes
as early as possible; warmup is a gap-free chain of N=512 matmuls keeping
the PE busy (and its clock governor ramping) until the first real data
lands; the last tile's PV t-chunks interleave into its exp chain one
chunk behind so the PE never drains waiting on ACT.
"""

import numpy as np

B, T, E, D = 8, 2048, 1024, 128
NT = 512                 # t-tile width (PSUM bank = 512 fp32)
N_TT = T // NT           # 4 t-tiles
N_TC = NT // 128         # 4 t-chunks per t-tile
N_EC = E // 128          # 8 e-chunks
N_EP = N_EC // 2         # 4 e-chunk pairs (DoubleRow K=256)
N_SC = T // 128          # 16 s-chunks
VS = 132                 # v_aug free stride (129 used)
T_FIX = 64               # rows recomputed exactly on host (causal only)
SCALE = float(1.0 / np.sqrt(D))

_cache: dict = {}


def _build(causal: bool):
    from contextlib import ExitStack
    import concourse.bass as bass
    import concourse.tile as tile
    from concourse import bacc, mybir
    from concourse.masks import make_identity

    f32 = mybir.dt.float32
    f16 = mybir.dt.float16
    f8 = mybir.dt.float8e4
    AF = mybir.ActivationFunctionType
    DR = mybir.MatmulPerfMode.DoubleRow

    nc = bacc.Bacc("TRN2", target_bir_lowering=False, debug=False,
                   num_devices=B)
    # x8L/W8L are host-pre-tiled to the exact SBUF image so DMA lines are
    # 1-4KB (fp8 through the natural [E,T] rearrange would mean 512B lines
    # and half the effective DMA bandwidth).
    x8L = nc.dram_tensor("x8L", (N_TT * 128, N_EC * NT), f8,
                         kind="ExternalInput").ap()
    Ws = {p: nc.dram_tensor(f"W{p}", (128, N_EC * 128), f8,
                            kind="ExternalInput").ap()
          for p in ("q", "k", "v", "vr")}
    bs = {p: nc.dram_tensor(f"b{p}", (D, 1), f32, kind="ExternalInput").ap()
          for p in "qkv"}
    out = nc.dram_tensor("out", (T, 129), f16, kind="ExternalOutput").ap()

    with tile.TileContext(nc) as tc, ExitStack() as ctx:
        consts = ctx.enter_context(tc.tile_pool(name="consts", bufs=1))
        xt_pool = ctx.enter_context(tc.tile_pool(name="xt", bufs=3))
        qT_pool = ctx.enter_context(tc.tile_pool(name="qT", bufs=2))
        vT_pool = ctx.enter_context(tc.tile_pool(name="vT", bufs=2))
        ex_pool = ctx.enter_context(tc.tile_pool(name="ex", bufs=3))
        outp = ctx.enter_context(tc.tile_pool(name="outp", bufs=4))
        ps_qkv = ctx.enter_context(tc.tile_pool(name="ps_qkv", bufs=2,
                                                space="PSUM"))
        ps_s = ctx.enter_context(tc.tile_pool(name="ps_s", bufs=3,
                                              space="PSUM"))
        ps_o = ctx.enter_context(tc.tile_pool(name="ps_o", bufs=2,
                                              space="PSUM"))
        ps_t = ctx.enter_context(tc.tile_pool(name="ps_t", bufs=1,
                                              space="PSUM"))

        # warmup stationary: gpsimd memset so the PE can start the instant
        # its queue preamble ends
        warm_t = consts.tile([128, NT], f16, tag="warm_t")
        nc.gpsimd.memset(warm_t[:], 0.0)

        # ---- input DMAs: ONE queue, exact consumption order ----
        w_t = {}

        def load_w(p):
            wt = consts.tile([128, N_EC * 128], f8, tag=f"w{p}")
            nc.sync.dma_start(wt[:], Ws[p])
            w_t[p] = wt

        def load_xt(jj, quarters=False):
            r0 = jj * 128
            xt = xt_pool.tile([128, N_EC * NT], f8, tag="xt")
            if quarters:
                # one DR e-pair per DMA so the first projection matmul can
                # fire after 128KB
                for qq in range(4):
                    c0, c1 = qq * 2 * NT, (qq + 1) * 2 * NT
                    nc.sync.dma_start(xt[:, c0:c1], x8L[r0:r0 + 128, c0:c1])
            else:
                nc.sync.dma_start(xt[:], x8L[r0:r0 + 128, :])
            return xt

        # q/k weights interleaved with tile-0 x quarters at half
        # granularity: the first projection matmul needs only 256KB
        wq_t = consts.tile([128, N_EC * 128], f8, tag="wq")
        wk_t = consts.tile([128, N_EC * 128], f8, tag="wk")
        w_t["q"], w_t["k"] = wq_t, wk_t
        xt0 = xt_pool.tile([128, N_EC * NT], f8, tag="xt")
        for h in range(2):
            wc0, wc1 = h * 4 * 128, (h + 1) * 4 * 128
            for p in "qk":
                nc.sync.dma_start(w_t[p][:, wc0:wc1], Ws[p][:, wc0:wc1])
            for qq in (2 * h, 2 * h + 1):
                c0, c1 = qq * 2 * NT, (qq + 1) * 2 * NT
                nc.sync.dma_start(xt0[:, c0:c1], x8L[0:128, c0:c1])
        load_w("v")
        load_w("vr")

        b_t = {}
        for p in "qkv":
            bt = consts.tile([128, 1], f32, tag=f"b{p}")
            nc.gpsimd.dma_start(bt[:], bs[p])
            b_t[p] = bt

        ident_h = consts.tile([128, 128], f16, tag="ident_h")
        make_identity(nc, ident_h[:])

        # PE warmup: continuous accumulation chain of N=512 matmuls on one
        # stationary — keeps the clock governor fed (no inter-matmul PSUM
        # drain) until the first real data lands.
        N_WARM = 7
        pw = ps_t.tile([128, NT], f32, tag="ps_t")
        for i in range(N_WARM):
            nc.tensor.matmul(pw[:], warm_t[:, 0:128], warm_t[:],
                             start=(i == 0), stop=(i == N_WARM - 1),
                             skip_group_check=True)

        masks_h = None
        if causal:
            # single lower-triangular (keep t>=s) 128x128 block
            masks_h = consts.tile([128, 128], f16, tag="masks_h")
            nc.gpsimd.memset(masks_h[:], 1.0)
            nc.gpsimd.affine_select(
                out=masks_h[:], in_=masks_h[:],
                compare_op=mybir.AluOpType.is_ge,
                fill=0.0, base=0, channel_multiplier=-1,
                pattern=[[1, 128]])

        kT_all = consts.tile([128, T], f16, tag="kT_all")
        v_all = consts.tile([128, N_SC * VS], f16, tag="v_all")
        nc.vector.memset(v_all[:], 1.0)  # keeps the ones column at VS*i+128

        qT_all = None
        if not causal:
            # full attention needs every t-tile's q resident before phase 2
            qT_all = consts.tile([128, T], f16, tag="qT_all")

        def pair(ap, c, w):
            # 3D AP [128, 2, w]: e-chunk pair (2c, 2c+1) for DoubleRow
            return ap[:, 2 * c * w:(2 * c + 2) * w].rearrange(
                "p (k n) -> p k n", k=2)

        def proj(p, xt, dest):
            # fp8 DoubleRow projection; V runs a second chain against the
            # weight residual into the same PSUM accumulation.
            chains = ("v", "vr") if p == "v" else (p,)
            n = len(chains) * N_EP
            ps = ps_qkv.tile([128, NT], f32, tag="ps_qkv")
            i = 0
            for w in chains:
                for c in range(N_EP):
                    nc.tensor.matmul(
                        ps[:], pair(w_t[w][:], c, 128), pair(xt[:], c, NT),
                        start=(i == 0), stop=(i == n - 1), perf_mode=DR)
                    i += 1
            nc.vector.tensor_scalar_add(dest, ps[:], b_t[p][:])

        def proj_qk_interleaved(xt, qdest, kdest):
            # tile 0 only: the x8 quarters arrive at the early-DMA crawl
            # rate, so consume each quarter twice (q and k chains) the
            # moment it lands instead of stalling between quarters.
            psq = ps_qkv.tile([128, NT], f32, tag="ps_qkv")
            psk = ps_qkv.tile([128, NT], f32, tag="ps_qkv")
            for c in range(N_EP):
                nc.tensor.matmul(
                    psq[:], pair(w_t["q"][:], c, 128), pair(xt[:], c, NT),
                    start=(c == 0), stop=(c == N_EP - 1), perf_mode=DR,
                    skip_group_check=True)
                nc.tensor.matmul(
                    psk[:], pair(w_t["k"][:], c, 128), pair(xt[:], c, NT),
                    start=(c == 0), stop=(c == N_EP - 1), perf_mode=DR,
                    skip_group_check=True)
            nc.vector.tensor_scalar_add(qdest, psq[:], b_t["q"][:])
            nc.vector.tensor_scalar_add(kdest, psk[:], b_t["k"][:])

        def score_chunk(j, qT, ex_all, i):
            # Diagonal s-chunk m: columns t_local < 128*m are never read by
            # PV, so compute only [128*m:NT] and mask the diagonal block.
            m = i - j * N_TC
            off = 128 * m if (causal and m > 0) else 0
            ps = ps_s.tile([128, NT], f32, tag="ps_s")
            nc.tensor.matmul(ps[:, off:NT],
                             kT_all[:, i * 128:(i + 1) * 128],
                             qT[:, off:NT], start=True, stop=True)
            ex = ex_all[:, i * NT + off:(i + 1) * NT]
            nc.scalar.activation(ex, ps[:, off:NT], AF.Exp, scale=SCALE)
            if causal and m >= 0:
                nc.gpsimd.tensor_mul(
                    ex_all[:, i * NT + off:i * NT + off + 128],
                    ex_all[:, i * NT + off:i * NT + off + 128],
                    masks_h[:])

        def scores_exp(j, qT, ex_all):
            n_sc = (j + 1) * N_TC if causal else N_SC
            for i in range(n_sc):
                score_chunk(j, qT, ex_all, i)

        def v_proj_transpose(j, xt):
            vT = vT_pool.tile([128, NT], f16, tag="vT")
            proj("v", xt, vT[:])
            for tch in range(N_TC):
                sc = j * N_TC + tch
                pt = ps_t.tile([128, 256], f16, tag="ps_t")
                nc.tensor.transpose(pt[:, 0:128],
                                    vT[:, tch * 128:(tch + 1) * 128],
                                    ident_h[:])
                nc.vector.tensor_copy(v_all[:, sc * VS:sc * VS + 128],
                                      pt[:, 0:128])

        def pv_chunk(j, ex_all, tch, last_one=False):
            # PV natural per t-chunk; denominator rides in column 128.
            # num||den copied out fp16; the host divides.
            t0 = j * NT
            tc_glob = j * N_TC + tch
            n_i = tc_glob + 1 if causal else N_SC
            po = ps_o.tile([128, VS], f32, tag="ps_o")
            for i in range(n_i):
                nc.tensor.matmul(
                    po[:, 0:129],
                    ex_all[:, i * NT + tch * 128:i * NT + (tch + 1) * 128],
                    v_all[:, i * VS:i * VS + 129],
                    start=(i == 0), stop=(i == n_i - 1),
                    skip_group_check=True)
            r0 = t0 + tch * 128
            ot = outp.tile([128, 132], f16, tag="ot")
            nc.vector.tensor_copy(ot[:, 0:129], po[:, 0:129])
            # gpsimd queue: keeps the sync queue free for input
            # descriptors; the final chunk rides the idle sync queue so
            # the two end-of-kernel DMA drains overlap
            eng = nc.sync if last_one else nc.gpsimd
            eng.dma_start(out[r0:r0 + 128, :], ot[:, 0:129])

        def pv_out(j, ex_all):
            for tch in range(N_TC):
                pv_chunk(j, ex_all, tch)

        xt_tiles = {0: xt0}
        if causal:
            prev = None
            for j in range(N_TT):
                t0 = j * NT
                last = j == N_TT - 1
                xt = xt_tiles.pop(j)
                qT = qT_pool.tile([128, NT], f16, tag="qT")
                if j == 0:
                    proj_qk_interleaved(xt, qT[:], kT_all[:, t0:t0 + NT])
                else:
                    proj("q", xt, qT[:])
                    proj("k", xt, kT_all[:, t0:t0 + NT])
                if j + 1 < N_TT:
                    xt_tiles[j + 1] = load_xt(j + 1)
                ex_all = ex_pool.tile([128, N_SC * NT], f16, tag="ex")

                # Interleave filler work (this tile's V projection and the
                # PREVIOUS tile's PV chunks) between score chunks: the exp
                # drain rate (~2x a score matmul) otherwise throttles the
                # in-order PE via ps_s buffer reuse.
                fillers = [("v", None)]
                if prev is not None:
                    fillers += [("pv", tch) for tch in range(N_TC)]
                n_sc = (j + 1) * N_TC
                sched = []
                si = 0
                if j == 0:
                    # scores need the q/k bias moves (DVE) after the
                    # chains; fill that latency with the V projection
                    sched.append(fillers.pop(0))
                for f in fillers:
                    take = 3 if not sched else 2
                    for _ in range(take):
                        if si < n_sc:
                            sched.append(("s", si))
                            si += 1
                    sched.append(f)
                tail_pv = []
                if last:
                    # own-tile PV chunks ride inside the score chain, one
                    # score chunk behind their diagonal dependency
                    while si < n_sc - 3:
                        sched.append(("s", si))
                        si += 1
                    for tch in range(N_TC):
                        if si < n_sc:
                            sched.append(("s", si))
                            si += 1
                        sched.append(("opv", tch))
                while si < n_sc:
                    sched.append(("s", si))
                    si += 1
                for kind, a in sched:
                    if kind == "s":
                        score_chunk(j, qT, ex_all, a)
                    elif kind == "v":
                        v_proj_transpose(j, xt)
                    elif kind == "pv":
                        pv_chunk(*prev, a)
                    else:
                        pv_chunk(j, ex_all, a, last_one=(a == N_TC - 1))
                prev = (j, ex_all)
        else:
            for j in range(N_TT):
                t0 = j * NT
                xt = xt_tiles.pop(j)
                proj("q", xt, qT_all[:, t0:t0 + NT])
                proj("k", xt, kT_all[:, t0:t0 + NT])
                v_proj_transpose(j, xt)
                if j + 1 < N_TT:
                    xt_tiles[j + 1] = load_xt(j + 1)
            for j in range(N_TT):
                ex_all = ex_pool.tile([128, N_SC * NT], f16, tag="ex")
                scores_exp(j, qT_all[:, j * NT:(j + 1) * NT], ex_all)
                pv_out(j, ex_all)

    nc.compile()
    return nc


def _get(causal: bool):
    if causal not in _cache:
        _cache[causal] = _build(causal)
    return _cache[causal]


def _f8(a):
    import ml_dtypes
    return np.ascontiguousarray(a.astype(ml_dtypes.float8_e4m3))


def _wtile(w8):
    # [E, D] -> SBUF image [128, N_EC*128]: row p, col c*128+d = W[c*128+p, d]
    return np.ascontiguousarray(
        w8.reshape(N_EC, 128, D).transpose(1, 0, 2).reshape(128, N_EC * D))


def _xtile(x8T):
    # [E, T] -> SBUF image [N_TT*128, N_EC*NT]:
    # row j*128+p, col c*NT+n = x8T[c*128+p, j*NT+n]
    return np.ascontiguousarray(
        x8T.reshape(N_EC, 128, N_TT, NT).transpose(2, 1, 0, 3)
        .reshape(N_TT * 128, N_EC * NT))


def _make_in_maps(x, Wq, bq, Wk, bk, Wv, bv):
    import ml_dtypes
    x = np.asarray(x, dtype=np.float32)
    Wv32 = np.asarray(Wv, np.float32)
    W8v = Wv32.astype(ml_dtypes.float8_e4m3)
    W8vr = _wtile(_f8(Wv32 - W8v.astype(np.float32)))
    Wq8 = _wtile(_f8(np.asarray(Wq, np.float32)))
    Wk8 = _wtile(_f8(np.asarray(Wk, np.float32)))
    W8v = _wtile(W8v)
    bq_c = np.ascontiguousarray(np.asarray(bq, np.float32).reshape(D, 1))
    bk_c = np.ascontiguousarray(np.asarray(bk, np.float32).reshape(D, 1))
    bv_c = np.ascontiguousarray(np.asarray(bv, np.float32).reshape(D, 1))
    in_maps = []
    for b in range(B):
        in_maps.append({
            "x8L": _xtile(_f8(x[b].T)),
            "Wq": Wq8, "Wk": Wk8, "Wv": W8v, "Wvr": W8vr,
            "bq": bq_c, "bk": bk_c, "bv": bv_c,
        })
    return in_maps


def _host_fix_rows(x, Wq, bq, Wk, bk, Wv, bv):
    # exact fp32 attention for the first T_FIX rows (small softmax support
    # means fp8 input-quantization error is not averaged away there)
    xf = np.asarray(x, np.float32)[:, :T_FIX]            # [B, T_FIX, E]
    q = xf @ np.asarray(Wq, np.float32) + np.asarray(bq, np.float32)
    k = xf @ np.asarray(Wk, np.float32) + np.asarray(bk, np.float32)
    v = xf @ np.asarray(Wv, np.float32) + np.asarray(bv, np.float32)
    s = np.einsum("btd,bsd->bts", q, k) / np.sqrt(np.float32(D))
    tri = np.tril(np.ones((T_FIX, T_FIX), bool))
    s = np.where(tri[None], s, -np.inf)
    s -= s.max(axis=-1, keepdims=True)
    e = np.exp(s)
    att = e / e.sum(axis=-1, keepdims=True)
    return np.einsum("bts,bsd->btd", att, v)             # [B, T_FIX, D]


def kernel(x, Wq, bq, Wk, bk, Wv, bv, mask, **_ignored):
    from concourse.bass_utils import run_bass_kernel_spmd

    causal = bool(np.asarray(mask).item()) if mask is not None else False
    nc = _get(causal)
    in_maps = _make_in_maps(x, Wq, bq, Wk, bk, Wv, bv)
    res = run_bass_kernel_spmd(nc, in_maps, core_ids=list(range(B)))
    nd = np.stack([res.results[b]["out"] for b in range(B)],
                  axis=0).astype(np.float32)             # [B, T, 129]
    o = nd[:, :, 0:128] / nd[:, :, 128:129]
    if causal:
        o[:, :T_FIX] = _host_fix_rows(x, Wq, bq, Wk, bk, Wv, bv)
    return np.ascontiguousarray(o, dtype=np.float32)


# revision 26
# speedup vs baseline: 1.0745x; 1.0745x over previous
"""Causal single-head attention on 8 TRN2 NeuronCores.

Problem: x[B=8,T=2048,E=1024] fp32, per-head Q/K/V projections (D=128) +
causal softmax attention. Sharding: data-parallel over batch B — one batch
element per core; Q/K/V weights replicated.

Per-core algorithm:

  x is fed pre-transposed as x8T [E, T] in fp8e4m3 so the contraction dim
  (E) lies on SBUF partitions and the input stream is 2MB instead of 4MB.
  1. qT/kT/vT [D, Tt] via fp8 DoubleRow matmuls: e-chunk PAIRS (K=256) per
     matmul at 0.5 cycles/row.  V adds a second accumulation chain against
     the fp8 weight residual rW8v so v == x8 @ Wv exactly up to fp8 input
     quantization of x.  Bias added during the PSUM->SBUF move (DVE
     tensor_scalar_add, fp16 out).
  2. v natural [s, D] chunks by PE-transposing vT 128x128 blocks, stored
     ones-augmented: v_aug [s, 129] with col 128 == 1.
  3. Scores TRANSPOSED in fp16: S^T tile [s 128, t 512] = kT_slice @ qT.
     exp via ACT (scale=1/sqrt(D)), fp16 out.  Causality: s-chunks above
     the diagonal skipped; diagonal chunks compute only the live column
     range; a 128x128 lower-triangle 0/1 multiply (gpsimd) masks the
     diagonal sub-block.
  4. PV natural per 128-row t-chunk: out_psum [t 128, 129] +=
     ex_slice-as-stationary @ v_aug_chunk; column 128 accumulates the
     softmax denominator via the ones column.
  5. No on-device normalize: numerator||denominator copied fp16 and DMA'd
     out [T, 129]; the host divides (and casts fp32).

The host also recomputes rows t < 64 exactly in fp32: for small softmax
support the fp8 x-quantization error is not averaged away, so those rows
are replaced with an exact host computation (cheap: 64 rows).

Edge scheduling: input DMAs ride one queue in exact consumption order
(W8q, x8-tile0 in halves, W8k, W8v+rW8v, x8-tile1..3) so each completes
as early as possible; warmup is a gap-free chain of N=512 matmuls keeping
the PE busy (and its clock governor ramping) until the first real data
lands; the last tile's PV t-chunks interleave into its exp chain one
chunk behind so the PE never drains waiting on ACT.
"""

import numpy as np

B, T, E, D = 8, 2048, 1024, 128
NT = 512                 # t-tile width (PSUM bank = 512 fp32)
N_TT = T // NT           # 4 t-tiles
N_TC = NT // 128         # 4 t-chunks per t-tile
N_EC = E // 128          # 8 e-chunks
N_EP = N_EC // 2         # 4 e-chunk pairs (DoubleRow K=256)
N_SC = T // 128          # 16 s-chunks
VS = 132                 # v_aug free stride (129 used)
T_FIX = 64               # rows recomputed exactly on host (causal only)
SCALE = float(1.0 / np.sqrt(D))

_cache: dict = {}


def _build(causal: bool):
    from contextlib import ExitStack
    import concourse.bass as bass
    import concourse.tile as tile
    from concourse import bacc, mybir
    from concourse.masks import make_identity

    f32 = mybir.dt.float32
    f16 = mybir.dt.float16
    f8 = mybir.dt.float8e4
    AF = mybir.ActivationFunctionType
    DR = mybir.MatmulPerfMode.DoubleRow

    nc = bacc.Bacc("TRN2", target_bir_lowering=False, debug=False,
                   num_devices=B)
    # x8L/W8L are host-pre-tiled to the exact SBUF image so DMA lines are
    # 1-4KB (fp8 through the natural [E,T] rearrange would mean 512B lines
    # and half the effective DMA bandwidth).
    x8L = nc.dram_tensor("x8L", (N_TT * 128, N_EC * NT), f8,
                         kind="ExternalInput").ap()
    Ws = {p: nc.dram_tensor(f"W{p}", (128, N_EC * 128), f8,
                            kind="ExternalInput").ap()
          for p in ("q", "k", "v", "vr")}
    bs = {p: nc.dram_tensor(f"b{p}", (D, 1), f32, kind="ExternalInput").ap()
          for p in "qkv"}
    out = nc.dram_tensor("out", (T, 129), f16, kind="ExternalOutput").ap()

    with tile.TileContext(nc) as tc, ExitStack() as ctx:
        consts = ctx.enter_context(tc.tile_pool(name="consts", bufs=1))
        xt_pool = ctx.enter_context(tc.tile_pool(name="xt", bufs=3))
        qT_pool = ctx.enter_context(tc.tile_pool(name="qT", bufs=2))
        vT_pool = ctx.enter_context(tc.tile_pool(name="vT", bufs=2))
        ex_pool = ctx.enter_context(tc.tile_pool(name="ex", bufs=3))
        outp = ctx.enter_context(tc.tile_pool(name="outp", bufs=4))
        ps_qkv = ctx.enter_context(tc.tile_pool(name="ps_qkv", bufs=2,
                                                space="PSUM"))
        ps_s = ctx.enter_context(tc.tile_pool(name="ps_s", bufs=3,
                                              space="PSUM"))
        ps_o = ctx.enter_context(tc.tile_pool(name="ps_o", bufs=2,
                                              space="PSUM"))
        ps_t = ctx.enter_context(tc.tile_pool(name="ps_t", bufs=1,
                                              space="PSUM"))

        # warmup stationary: gpsimd memset so the PE can start the instant
        # its queue preamble ends
        warm_t = consts.tile([128, NT], f16, tag="warm_t")
        nc.gpsimd.memset(warm_t[:], 0.0)

        # ---- input DMAs: ONE queue, exact consumption order ----
        w_t = {}

        def load_w(p):
            wt = consts.tile([128, N_EC * 128], f8, tag=f"w{p}")
            nc.sync.dma_start(wt[:], Ws[p])
            w_t[p] = wt

        def load_xt(jj, quarters=False):
            r0 = jj * 128
            xt = xt_pool.tile([128, N_EC * NT], f8, tag="xt")
            if quarters:
                # one DR e-pair per DMA so the first projection matmul can
                # fire after 128KB
                for qq in range(4):
                    c0, c1 = qq * 2 * NT, (qq + 1) * 2 * NT
                    nc.sync.dma_start(xt[:, c0:c1], x8L[r0:r0 + 128, c0:c1])
            else:
                nc.sync.dma_start(xt[:], x8L[r0:r0 + 128, :])
            return xt

        load_w("q")
        load_w("k")
        xt0 = load_xt(0, quarters=True)
        load_w("v")
        load_w("vr")

        b_t = {}
        for p in "qkv":
            bt = consts.tile([128, 1], f32, tag=f"b{p}")
            nc.gpsimd.dma_start(bt[:], bs[p])
            b_t[p] = bt

        ident_h = consts.tile([128, 128], f16, tag="ident_h")
        make_identity(nc, ident_h[:])

        # PE warmup: continuous accumulation chain of N=512 matmuls on one
        # stationary.  Long on purpose: the clock governor grants full
        # speed ~3.2us after the PE's LAST idle gap, so real work must not
        # start until its data is fully resident — the warmup covers the
        # whole input-DMA window (~13us) and the kernel then runs gap-free
        # at 2.4GHz, which beats starting "real" matmuls early at the
        # ungranted 0.8GHz clock and resetting the governor on every
        # DMA-starved stall.
        N_WARM = 12
        pw = ps_t.tile([128, NT], f32, tag="ps_t")
        for i in range(N_WARM):
            nc.tensor.matmul(pw[:], warm_t[:, 0:128], warm_t[:],
                             start=(i == 0), stop=(i == N_WARM - 1),
                             skip_group_check=True)

        masks_h = None
        if causal:
            # single lower-triangular (keep t>=s) 128x128 block
            masks_h = consts.tile([128, 128], f16, tag="masks_h")
            nc.gpsimd.memset(masks_h[:], 1.0)
            nc.gpsimd.affine_select(
                out=masks_h[:], in_=masks_h[:],
                compare_op=mybir.AluOpType.is_ge,
                fill=0.0, base=0, channel_multiplier=-1,
                pattern=[[1, 128]])

        kT_all = consts.tile([128, T], f16, tag="kT_all")
        v_all = consts.tile([128, N_SC * VS], f16, tag="v_all")
        nc.vector.memset(v_all[:], 1.0)  # keeps the ones column at VS*i+128

        qT_all = None
        if not causal:
            # full attention needs every t-tile's q resident before phase 2
            qT_all = consts.tile([128, T], f16, tag="qT_all")

        def pair(ap, c, w):
            # 3D AP [128, 2, w]: e-chunk pair (2c, 2c+1) for DoubleRow
            return ap[:, 2 * c * w:(2 * c + 2) * w].rearrange(
                "p (k n) -> p k n", k=2)

        def proj(p, xt, dest):
            # fp8 DoubleRow projection; V runs a second chain against the
            # weight residual into the same PSUM accumulation.
            chains = ("v", "vr") if p == "v" else (p,)
            n = len(chains) * N_EP
            ps = ps_qkv.tile([128, NT], f32, tag="ps_qkv")
            i = 0
            for w in chains:
                for c in range(N_EP):
                    nc.tensor.matmul(
                        ps[:], pair(w_t[w][:], c, 128), pair(xt[:], c, NT),
                        start=(i == 0), stop=(i == n - 1), perf_mode=DR)
                    i += 1
            nc.vector.tensor_scalar_add(dest, ps[:], b_t[p][:])

        def proj_qk_interleaved(xt, qdest, kdest):
            # tile 0 only: the x8 quarters arrive at the early-DMA crawl
            # rate, so consume each quarter twice (q and k chains) the
            # moment it lands instead of stalling between quarters.
            psq = ps_qkv.tile([128, NT], f32, tag="ps_qkv")
            psk = ps_qkv.tile([128, NT], f32, tag="ps_qkv")
            for c in range(N_EP):
                nc.tensor.matmul(
                    psq[:], pair(w_t["q"][:], c, 128), pair(xt[:], c, NT),
                    start=(c == 0), stop=(c == N_EP - 1), perf_mode=DR,
                    skip_group_check=True)
                nc.tensor.matmul(
                    psk[:], pair(w_t["k"][:], c, 128), pair(xt[:], c, NT),
                    start=(c == 0), stop=(c == N_EP - 1), perf_mode=DR,
                    skip_group_check=True)
            nc.vector.tensor_scalar_add(qdest, psq[:], b_t["q"][:])
            nc.vector.tensor_scalar_add(kdest, psk[:], b_t["k"][:])

        def score_chunk(j, qT, ex_all, i):
            # Diagonal s-chunk m: columns t_local < 128*m are never read by
            # PV, so compute only [128*m:NT] and mask the diagonal block.
            m = i - j * N_TC
            off = 128 * m if (causal and m > 0) else 0
            ps = ps_s.tile([128, NT], f32, tag="ps_s")
            nc.tensor.matmul(ps[:, off:NT],
                             kT_all[:, i * 128:(i + 1) * 128],
                             qT[:, off:NT], start=True, stop=True)
            ex = ex_all[:, i * NT + off:(i + 1) * NT]
            nc.scalar.activation(ex, ps[:, off:NT], AF.Exp, scale=SCALE)
            if causal and m >= 0:
                # DVE, not gpsimd: on gpsimd these queue behind output-DMA
                # descriptor writes and stall the tail PV chains
                nc.vector.tensor_mul(
                    ex_all[:, i * NT + off:i * NT + off + 128],
                    ex_all[:, i * NT + off:i * NT + off + 128],
                    masks_h[:])

        def scores_exp(j, qT, ex_all):
            n_sc = (j + 1) * N_TC if causal else N_SC
            for i in range(n_sc):
                score_chunk(j, qT, ex_all, i)

        def v_proj_transpose(j, xt):
            vT = vT_pool.tile([128, NT], f16, tag="vT")
            proj("v", xt, vT[:])
            for tch in range(N_TC):
                sc = j * N_TC + tch
                pt = ps_t.tile([128, 256], f16, tag="ps_t")
                nc.tensor.transpose(pt[:, 0:128],
                                    vT[:, tch * 128:(tch + 1) * 128],
                                    ident_h[:])
                nc.vector.tensor_copy(v_all[:, sc * VS:sc * VS + 128],
                                      pt[:, 0:128])

        def pv_chunk(j, ex_all, tch, last_one=False):
            # PV natural per t-chunk; denominator rides in column 128.
            # num||den copied out fp16; the host divides.
            t0 = j * NT
            tc_glob = j * N_TC + tch
            n_i = tc_glob + 1 if causal else N_SC
            po = ps_o.tile([128, VS], f32, tag="ps_o")
            for i in range(n_i):
                nc.tensor.matmul(
                    po[:, 0:129],
                    ex_all[:, i * NT + tch * 128:i * NT + (tch + 1) * 128],
                    v_all[:, i * VS:i * VS + 129],
                    start=(i == 0), stop=(i == n_i - 1),
                    skip_group_check=True)
            r0 = t0 + tch * 128
            ot = outp.tile([128, 132], f16, tag="ot")
            nc.vector.tensor_copy(ot[:, 0:129], po[:, 0:129])
            # gpsimd queue: keeps the sync queue free for input
            # descriptors; the final chunk rides the idle sync queue so
            # the two end-of-kernel DMA drains overlap
            eng = nc.sync if last_one else nc.gpsimd
            eng.dma_start(out[r0:r0 + 128, :], ot[:, 0:129])

        def pv_out(j, ex_all):
            for tch in range(N_TC):
                pv_chunk(j, ex_all, tch)

        xt_tiles = {0: xt0}
        if causal:
            prev = None
            for j in range(N_TT):
                t0 = j * NT
                last = j == N_TT - 1
                xt = xt_tiles.pop(j)
                qT = qT_pool.tile([128, NT], f16, tag="qT")
                if j == 0:
                    proj_qk_interleaved(xt, qT[:], kT_all[:, t0:t0 + NT])
                else:
                    proj("q", xt, qT[:])
                    proj("k", xt, kT_all[:, t0:t0 + NT])
                if j + 1 < N_TT:
                    xt_tiles[j + 1] = load_xt(j + 1)
                ex_all = ex_pool.tile([128, N_SC * NT], f16, tag="ex")

                # Interleave filler work (this tile's V projection and the
                # PREVIOUS tile's PV chunks) between score chunks: the exp
                # drain rate (~2x a score matmul) otherwise throttles the
                # in-order PE via ps_s buffer reuse.
                fillers = [("v", None)]
                if prev is not None:
                    fillers += [("pv", tch) for tch in range(N_TC)]
                n_sc = (j + 1) * N_TC
                sched = []
                si = 0
                if j == 0:
                    # scores need the q/k bias moves (DVE) after the
                    # chains; fill that latency with the V projection
                    sched.append(fillers.pop(0))
                for f in fillers:
                    take = 3 if not sched else 2
                    for _ in range(take):
                        if si < n_sc:
                            sched.append(("s", si))
                            si += 1
                    sched.append(f)
                tail_pv = []
                if last:
                    # own-tile PV chunks ride inside the score chain, one
                    # score chunk behind their diagonal dependency
                    while si < n_sc - 3:
                        sched.append(("s", si))
                        si += 1
                    for tch in range(N_TC):
                        if si < n_sc:
                            sched.append(("s", si))
                            si += 1
                        sched.append(("opv", tch))
                while si < n_sc:
                    sched.append(("s", si))
                    si += 1
                for kind, a in sched:
                    if kind == "s":
                        score_chunk(j, qT, ex_all, a)
                    elif kind == "v":
                        v_proj_transpose(j, xt)
                    elif kind == "pv":
                        pv_chunk(*prev, a)
                    else:
                        pv_chunk(j, ex_all, a, last_one=(a == N_TC - 1))
                prev = (j, ex_all)
        else:
            for j in range(N_TT):
                t0 = j * NT
                xt = xt_tiles.pop(j)
                proj("q", xt, qT_all[:, t0:t0 + NT])
                proj("k", xt, kT_all[:, t0:t0 + NT])
                v_proj_transpose(j, xt)
                if j + 1 < N_TT:
                    xt_tiles[j + 1] = load_xt(j + 1)
            for j in range(N_TT):
                ex_all = ex_pool.tile([128, N_SC * NT], f16, tag="ex")
                scores_exp(j, qT_all[:, j * NT:(j + 1) * NT], ex_all)
                pv_out(j, ex_all)

    nc.compile()
    return nc


def _get(causal: bool):
    if causal not in _cache:
        _cache[causal] = _build(causal)
    return _cache[causal]


def _f8(a):
    import ml_dtypes
    return np.ascontiguousarray(a.astype(ml_dtypes.float8_e4m3))


def _wtile(w8):
    # [E, D] -> SBUF image [128, N_EC*128]: row p, col c*128+d = W[c*128+p, d]
    return np.ascontiguousarray(
        w8.reshape(N_EC, 128, D).transpose(1, 0, 2).reshape(128, N_EC * D))


def _xtile(x8T):
    # [E, T] -> SBUF image [N_TT*128, N_EC*NT]:
    # row j*128+p, col c*NT+n = x8T[c*128+p, j*NT+n]
    return np.ascontiguousarray(
        x8T.reshape(N_EC, 128, N_TT, NT).transpose(2, 1, 0, 3)
        .reshape(N_TT * 128, N_EC * NT))


def _make_in_maps(x, Wq, bq, Wk, bk, Wv, bv):
    import ml_dtypes
    x = np.asarray(x, dtype=np.float32)
    Wv32 = np.asarray(Wv, np.float32)
    W8v = Wv32.astype(ml_dtypes.float8_e4m3)
    W8vr = _wtile(_f8(Wv32 - W8v.astype(np.float32)))
    Wq8 = _wtile(_f8(np.asarray(Wq, np.float32)))
    Wk8 = _wtile(_f8(np.asarray(Wk, np.float32)))
    W8v = _wtile(W8v)
    bq_c = np.ascontiguousarray(np.asarray(bq, np.float32).reshape(D, 1))
    bk_c = np.ascontiguousarray(np.asarray(bk, np.float32).reshape(D, 1))
    bv_c = np.ascontiguousarray(np.asarray(bv, np.float32).reshape(D, 1))
    in_maps = []
    for b in range(B):
        in_maps.append({
            "x8L": _xtile(_f8(x[b].T)),
            "Wq": Wq8, "Wk": Wk8, "Wv": W8v, "Wvr": W8vr,
            "bq": bq_c, "bk": bk_c, "bv": bv_c,
        })
    return in_maps


def _host_fix_rows(x, Wq, bq, Wk, bk, Wv, bv):
    # exact fp32 attention for the first T_FIX rows (small softmax support
    # means fp8 input-quantization error is not averaged away there)
    xf = np.asarray(x, np.float32)[:, :T_FIX]            # [B, T_FIX, E]
    q = xf @ np.asarray(Wq, np.float32) + np.asarray(bq, np.float32)
    k = xf @ np.asarray(Wk, np.float32) + np.asarray(bk, np.float32)
    v = xf @ np.asarray(Wv, np.float32) + np.asarray(bv, np.float32)
    s = np.einsum("btd,bsd->bts", q, k) / np.sqrt(np.float32(D))
    tri = np.tril(np.ones((T_FIX, T_FIX), bool))
    s = np.where(tri[None], s, -np.inf)
    s -= s.max(axis=-1, keepdims=True)
    e = np.exp(s)
    att = e / e.sum(axis=-1, keepdims=True)
    return np.einsum("bts,bsd->btd", att, v)             # [B, T_FIX, D]


def kernel(x, Wq, bq, Wk, bk, Wv, bv, mask, **_ignored):
    from concourse.bass_utils import run_bass_kernel_spmd

    causal = bool(np.asarray(mask).item()) if mask is not None else False
    nc = _get(causal)
    in_maps = _make_in_maps(x, Wq, bq, Wk, bk, Wv, bv)
    res = run_bass_kernel_spmd(nc, in_maps, core_ids=list(range(B)))
    nd = np.stack([res.results[b]["out"] for b in range(B)],
                  axis=0).astype(np.float32)             # [B, T, 129]
    o = nd[:, :, 0:128] / nd[:, :, 128:129]
    if causal:
        o[:, :T_FIX] = _host_fix_rows(x, Wq, bq, Wk, bk, Wv, bv)
    return np.ascontiguousarray(o, dtype=np.float32)
